# revision 9
# baseline (speedup 1.0000x reference)
"""Capsule routing softmax+matvec+squash kernel for 8 Trainium2 NeuronCores.

Problem (hardcoded shapes):
    u_hat: [8192] f32
    b:     [4096, 8192] f32
    c = softmax(b, axis=-1); s = c @ u_hat            -> [4096]
    v = |s|^2 * s / ((1+|s|^2) * |s|)                 -> [4096]

Sharding: b row-wise across 8 cores (512 rows each), u_hat replicated.

v3 design. The kernel is DMA-pool bound (16 engines x ~22.5 GB/s ~=
360 GB/s per core -> 4.19 MiB of int8 codes stream in ~11.6 us), and
the exp work is spread over THREE engines to fit under the stream pace
(ACT ~0.97 ns/col, DVE tensor_scalar ~0.53 ns/col = its 2x mode cap
for 1-byte sources, GpSimd ~2 ns/col):

  * j-columns are sorted by |u| and tiered:
      - 'a' (top |u|, NA groups of 128): ACT true exp -> bf16.
      - 's' (mid, NS groups): DVE Schraudolph bf16 bit-exp.
      - 'f'/'p' (bottom, NF groups): Schraudolph fp8-e4m3 bit-exp on
        DVE ('f') or GpSimd ('p'); ln|u| is folded into b on the host
        so the fp8 value is exp(b)*|u| and the weights are exact signs
        (num) / 1-over-u capped at 240, TRN2's e4m3 max (den).
  * HOST-side inverse-optimal quantization: the int8 code q is chosen
    so the DEVICE-decoded value is log-nearest to exp(b), merging the
    int8 and mantissa quantizations into one error.
  * PE: bf16 groups cost 512 cols (213 ns); fp8 groups go in PAIRS via
    DoubleRow matmuls (2 j's per partition-cycle). One PSUM [2, 512]:
        row 0 = den = sum exp(b), row 1 = num = sum exp(b)*u.
  * DMA: 9 large triggers on the sync HWDGE queue (trigger issue is
    ~0.6 us each on the queue engine); each trigger carries 2-3 exp
    SEGMENTS of different modes. a/p segment matmuls are DEFERRED 2/3
    triggers so the slow exp engines' latency hides behind the stream.
  * PE warm-up dummies burn the idle pre-stream window (DVFS ramp).

Host: s = num/den, global squash (O(4096) scalar work).
"""

import os
from contextlib import ExitStack

import numpy as np

J = 8192
CAPS = 4096
N_CORES = 8
R = CAPS // N_CORES              # 512 rows (capsules) per core
JG = J // 128                    # 64 j-groups of 128

# Two-level schedule: "|"-separated DMA triggers, each a comma list of
# exp segments "<mode><groups>".
#   a = ACT true exp (bf16)        s = DVE Schraudolph bf16
#   f = DVE Schraudolph e4m3       p = GpSimd Schraudolph e4m3
_SCHED = os.environ.get(
    "KERNEL_SCHED",
    "a2,s2|p2,a4,f2|p2,a4,f2|p2,a4,f2|p2,a2,s4|p2,a4,f2|p2,s4,f2|f6|s2,f4")
SCHED = [[(t[0], int(t[1:])) for t in trig.split(",")]
         for trig in _SCHED.split("|")]
SEGS = [seg for trig in SCHED for seg in trig]
NA = sum(g for m, g in SEGS if m == "a")
NS = sum(g for m, g in SEGS if m == "s")
NF = sum(g for m, g in SEGS if m in "fp")
DEFER = {"a": 2, "p": 3, "s": 0, "f": 0}

S8 = float(os.environ.get("KERNEL_S8", str(5.45 / 127)))
K1_16 = 128.0 / 0.6931471805599453     # 2^7 / ln2  (bf16 bits per unit b)
C16 = 7.0
K2_16 = 127.0 * 128.0 - C16
K1_8 = 8.0 / 0.6931471805599453        # 2^3 / ln2  (e4m3 bits per unit b)
C8 = 0.438
K2_8 = 7.0 * 8.0 - C8
QF_MIN = -112                           # smallest f-tier code: bits(q) >= 0

_CACHED = {}


def _check_cfg():
    assert NA + NS + NF == JG
    assert all(g % 2 == 0 for m, g in SEGS if m in "fp")
    assert NF % 2 == 0


def _seg_meta():
    """Per segment: (trig_idx, mode, groups, col_off, c16_base, p8_base)."""
    meta, off, c16, p8 = [], 0, 0, 0
    for t, trig in enumerate(SCHED):
        for m, g in trig:
            meta.append((t, m, g, off, c16, p8))
            off += g * R
            if m in "as":
                c16 += g
            else:
                p8 += g // 2
    assert off == JG * R and c16 == NA + NS and p8 == NF // 2
    return meta


def _emission_order(meta):
    """Segment MM emission order: s/f inline, a/p deferred by trigger."""
    order, pend = [], []
    n_trig = len(SCHED)
    for t in range(n_trig):
        for i, (ti, m, g, off, c16b, p8b) in enumerate(meta):
            if ti == t:
                if m in "sf":
                    order.append(i)
                else:
                    pend.append(i)
        for i in list(pend):
            ti, m = meta[i][0], meta[i][1]
            if ti <= t - DEFER[m]:
                order.append(i)
                pend.remove(i)
    order.extend(pend)
    assert sorted(order) == list(range(len(meta)))
    return order


def _build_bass():
    import concourse.bass as bass
    import concourse.tile as tile
    from concourse import bacc, mybir

    _check_cfg()
    f32 = mybir.dt.float32
    bf16 = mybir.dt.bfloat16
    i16 = mybir.dt.int16
    i8 = mybir.dt.int8
    f8 = mybir.dt.float8e4

    nc = bacc.Bacc("TRN2", target_bir_lowering=False, debug=False,
                   num_devices=N_CORES)

    bt8_ap = nc.dram_tensor("bt8", [128, JG * R], i8,
                            kind="ExternalInput").ap()
    w16_ap = nc.dram_tensor("w16", [128, 2 * (NA + NS)], bf16,
                            kind="ExternalInput").ap()
    # DoubleRow ldweights needs the pair-dim stride %16B == 0: pad
    # each weight column to 16 bytes.
    w8_ap = nc.dram_tensor("w8", [128, NF // 2, 2, 16], f8,
                           kind="ExternalInput").ap()
    out_ap = nc.dram_tensor("nd_out", [2, R], f32,
                            kind="ExternalOutput").ap()

    meta = _seg_meta()
    order = _emission_order(meta)
    total_mm = (NA + NS) + NF // 2

    with tile.TileContext(nc) as tc, ExitStack() as ctx:
        # every pool holds ALL its tiles at once (total SBUF ~90 KiB)
        # so buffer rotation never gates the DMA stream or an exp engine
        n_trig = len(SCHED)
        n16 = sum(1 for m, g in SEGS if m in "as")
        n8 = sum(1 for m, g in SEGS if m in "fp")
        bpool = ctx.enter_context(tc.tile_pool(name="bl", bufs=n_trig))
        e16p = ctx.enter_context(tc.tile_pool(name="e16", bufs=n16))
        e8p = ctx.enter_context(tc.tile_pool(name="e8", bufs=n8))
        wpool = ctx.enter_context(tc.tile_pool(name="w", bufs=1))
        opool = ctx.enter_context(tc.tile_pool(name="o", bufs=1))
        psum = ctx.enter_context(
            tc.tile_pool(name="ps", bufs=1, space=bass.MemorySpace.PSUM))

        # w on the vector HWDGE queue (DVE idle early): keeps ACT's
        # queue clear so its table load finishes before the first a-data
        # lands.
        w16_sb = wpool.tile([128, 2 * (NA + NS)], bf16)
        nc.scalar.dma_start(w16_sb[:], w16_ap[:, :])
        w8_sb = wpool.tile([128, NF // 2, 2, 16], f8)
        nc.scalar.dma_start(w8_sb[:], w8_ap[:, :, :, :])

        # PE DVFS warm-up in the idle pre-stream window.
        wu = int(os.environ.get("KERNEL_WARMUP_MM", "16"))
        d_ps = dummy = None
        if wu:
            dpool = ctx.enter_context(tc.tile_pool(name="dmy", bufs=1))
            dps = ctx.enter_context(
                tc.tile_pool(name="dps", bufs=1,
                             space=bass.MemorySpace.PSUM))
            dummy = dpool.tile([128, 256], bf16)
            nc.vector.memset(dummy[:], 0.0)
            d_ps = dps.tile([2, 256], f32)
            for _ in range(wu):
                nc.tensor.matmul(d_ps[:, :], dummy[:, 0:2], dummy[:, :],
                                 start=True, stop=True)

        nd_ps = psum.tile([2, R], f32)

        # One DMA trigger per trigger-group; exp per segment.
        e_tiles = {}
        seg_i = 0
        for t, trig in enumerate(SCHED):
            tg = sum(g for _, g in trig)
            t_off = meta[seg_i][3]
            bt = bpool.tile([128, tg * R], i8, tag="bl")
            nc.sync.dma_start(bt[:], bt8_ap[:, t_off:t_off + tg * R])
            b_off = 0
            for m, g in trig:
                if m in "fp":
                    et = e8p.tile([128, g, R], f8, tag="e8")
                else:
                    et = e16p.tile([128, g, R], bf16, tag="e16")
                e_tiles[seg_i] = et
                # small-first halves so the in-order PE starts sooner
                if m in "sf" and g >= 4:
                    halves = [(0, 2), (2, g)]
                else:
                    halves = [(0, g)]
                for lo, hi in halves:
                    e2 = et[:, lo:hi, :].opt()
                    b2 = bt[:, (b_off + lo) * R:(b_off + hi) * R]
                    if m == "a":
                        nc.scalar.activation(
                            e2, b2, mybir.ActivationFunctionType.Exp,
                            scale=S8)
                    elif m == "s":
                        nc.vector.tensor_scalar(
                            out=e2.bitcast(i16), in0=b2,
                            scalar1=S8 * K1_16, scalar2=K2_16,
                            op0=mybir.AluOpType.mult,
                            op1=mybir.AluOpType.add)
                    elif m == "f":
                        nc.vector.tensor_scalar(
                            out=e2.bitcast(i8), in0=b2,
                            scalar1=S8 * K1_8, scalar2=K2_8,
                            op0=mybir.AluOpType.mult,
                            op1=mybir.AluOpType.add)
                    else:
                        nc.gpsimd.tensor_scalar(
                            out=e2.bitcast(i8), in0=b2,
                            scalar1=S8 * K1_8, scalar2=K2_8,
                            op0=mybir.AluOpType.mult,
                            op1=mybir.AluOpType.add)
                b_off += g
                seg_i += 1

        # Matmuls in deferred emission order, one PSUM accumulation
        # group: psum[0,:] += 1-col (den), psum[1,:] += u-col (num).
        mm_idx = 0
        for si in order:
            ti, m, g, off, c16b, p8b = meta[si]
            et = e_tiles[si]
            if m in "as":
                for i in range(g):
                    c = c16b + i
                    nc.tensor.matmul(
                        nd_ps[:, :], w16_sb[:, 2 * c:2 * c + 2],
                        et[:, i, :],
                        start=(mm_idx == 0), stop=(mm_idx == total_mm - 1))
                    mm_idx += 1
            else:
                for i in range(g // 2):
                    p = p8b + i
                    nc.tensor.matmul(
                        nd_ps[:, :], w8_sb[:, p, :, 0:2],
                        et[:, 2 * i:2 * i + 2, :],
                        perf_mode=mybir.MatmulPerfMode.DoubleRow,
                        start=(mm_idx == 0), stop=(mm_idx == total_mm - 1))
                    mm_idx += 1
            if dummy is not None and mm_idx <= 30:
                nc.tensor.matmul(d_ps[:, :], dummy[:, 0:2],
                                 dummy[:, :], start=True, stop=True)
        assert mm_idx == total_mm

        # DMA cannot read PSUM; bounce through SBUF on ACT (idle at the
        # tail; DVE is the loaded engine), output on the sync queue.
        nd_sb = opool.tile([2, R], f32)
        nc.scalar.activation(nd_sb[:], nd_ps[:],
                             mybir.ActivationFunctionType.Identity,
                             scale=1.0)
        nc.sync.dma_start(out_ap[:, :], nd_sb[:])

    nc.compile()
    return nc


def _get_nc():
    if "nc" not in _CACHED:
        _CACHED["nc"] = _build_bass()
    return _CACHED["nc"]


def _img(x):
    """[R, n*128] slot-major -> [128, n*R] partition-major image."""
    r, w = x.shape
    n = w // 128
    return np.ascontiguousarray(
        x.T.reshape(n, 128, r).transpose(1, 0, 2).reshape(128, n * r))


def _grids():
    """Device-decoded value grids for inverse-optimal quantization."""
    import ml_dtypes
    bf16 = ml_dtypes.bfloat16
    e4m3 = ml_dtypes.float8_e4m3fn
    q = np.arange(-127, 128, dtype=np.float32)
    bits16 = np.rint(q * np.float32(S8 * K1_16)
                     + np.float32(K2_16)).astype(np.int16)
    v16 = bits16.view(bf16).astype(np.float64)
    qf = np.arange(QF_MIN, 128, dtype=np.float32)
    bits8 = np.rint(qf * np.float32(S8 * K1_8)
                    + np.float32(K2_8)).astype(np.int8)
    assert bits8.min() >= 0
    v8 = bits8.view(e4m3).astype(np.float64)
    return v16, v8


def _inv_quant(x, v, q0):
    """Per-element code q minimizing log-distance of decode v[q-q0] to
    exp(x). v must be non-decreasing."""
    lv = np.log(np.maximum(v, 1e-300)).astype(np.float32)
    mid = (lv[1:] + lv[:-1]) * np.float32(0.5)
    idx = np.searchsorted(mid, x.astype(np.float32))
    return (idx + q0).astype(np.int8)


def kernel(u_hat: np.ndarray, b: np.ndarray) -> np.ndarray:
    import ml_dtypes
    from concourse import bass_utils

    bf16 = ml_dtypes.bfloat16
    e4m3 = ml_dtypes.float8_e4m3fn
    assert u_hat.shape == (J,) and b.shape == (CAPS, J)
    nc = _get_nc()

    order_u = np.argsort(np.abs(u_hat), kind="stable")
    pool_f = list(order_u[:NF * 128])          # f and p segments
    pool_s = list(order_u[NF * 128:(NF + NS) * 128])
    pool_a = list(order_u[(NF + NS) * 128:])
    pools = {"a": pool_a, "s": pool_s, "f": pool_f, "p": pool_f}

    # slot order = image order = segment-major per SCHED
    jslot = np.empty(J, np.int64)
    pos = 0
    for m, g in SEGS:
        n = g * 128
        jslot[pos:pos + n] = pools[m][:n]
        del pools[m][:n]
        pos += n
    assert pos == J and not pool_f and not pool_s and not pool_a

    v16, v8 = _grids()
    q_all = np.empty((CAPS, J), np.int8)
    w16 = np.empty((128, 2 * (NA + NS)), dtype=bf16)
    w8 = np.zeros((128, NF // 2, 2, 16), dtype=e4m3)
    pos = 0
    c16 = p8 = 0
    for m, g in SEGS:
        n = g * 128
        js = jslot[pos:pos + n]
        cols = slice(pos, pos + n)
        if m == "a":
            q_all[:, cols] = np.clip(
                np.rint(b[:, js] / S8), -127, 127).astype(np.int8)
        elif m == "s":
            q_all[:, cols] = _inv_quant(b[:, js], v16, -127)
        else:
            au = np.abs(u_hat[js]).astype(np.float64)
            q_all[:, cols] = _inv_quant(
                b[:, js] + np.log(np.maximum(au, 1e-300))[None, :],
                v8, QF_MIN)
        if m in "as":
            uu = u_hat[js].astype(bf16).reshape(g, 128)
            for i in range(g):
                w16[:, 2 * (c16 + i)] = 1.0
                w16[:, 2 * (c16 + i) + 1] = uu[i]
            c16 += g
        else:
            au = np.abs(u_hat[js]).reshape(g, 128)
            sg = np.where(u_hat[js] >= 0, 1.0, -1.0).reshape(g, 128)
            wd = np.minimum(1.0 / np.maximum(au, 1e-30), 240.0)
            for i in range(g // 2):
                w8[:, p8 + i, 0, 0] = wd[2 * i].astype(e4m3)
                w8[:, p8 + i, 0, 1] = sg[2 * i].astype(e4m3)
                w8[:, p8 + i, 1, 0] = wd[2 * i + 1].astype(e4m3)
                w8[:, p8 + i, 1, 1] = sg[2 * i + 1].astype(e4m3)
            p8 += g // 2
        pos += n

    in_maps = []
    for i in range(N_CORES):
        rows = slice(i * R, (i + 1) * R)
        in_maps.append({"bt8": _img(q_all[rows]), "w16": w16, "w8": w8})

    res = bass_utils.run_bass_kernel_spmd(
        nc, in_maps, core_ids=list(range(N_CORES)),
        trace=bool(int(os.environ.get("KERNEL_TRACE", "0"))),
    )
    _CACHED["last_results"] = res

    nd = np.stack([r["nd_out"] for r in res.results]).astype(np.float64)
    den = nd[:, 0, :].reshape(-1)
    num = nd[:, 1, :].reshape(-1)
    s = num / den

    s_mag_sq = np.sum(s * s)
    s_mag = np.sqrt(s_mag_sq)
    v = s_mag_sq * s / ((1.0 + s_mag_sq) * s_mag)
    return v.astype(np.float32)


# revision 10
# speedup vs baseline: 1.0249x; 1.0249x over previous
"""Capsule routing softmax+matvec+squash kernel for 8 Trainium2 NeuronCores.

Problem (hardcoded shapes):
    u_hat: [8192] f32
    b:     [4096, 8192] f32
    c = softmax(b, axis=-1); s = c @ u_hat            -> [4096]
    v = |s|^2 * s / ((1+|s|^2) * |s|)                 -> [4096]

Sharding: b row-wise across 8 cores (512 rows each), u_hat replicated.

v3 design. The kernel is DMA-pool bound (16 engines x ~22.5 GB/s ~=
360 GB/s per core -> 4.19 MiB of int8 codes stream in ~11.6 us), and
the exp work is spread over THREE engines to fit under the stream pace
(ACT ~0.97 ns/col, DVE tensor_scalar ~0.53 ns/col = its 2x mode cap
for 1-byte sources, GpSimd ~2 ns/col):

  * j-columns are sorted by |u| and tiered:
      - 'a' (top |u|, NA groups of 128): ACT true exp -> bf16.
      - 's' (mid, NS groups): DVE Schraudolph bf16 bit-exp.
      - 'f'/'p' (bottom, NF groups): Schraudolph fp8-e4m3 bit-exp on
        DVE ('f') or GpSimd ('p'); ln|u| is folded into b on the host
        so the fp8 value is exp(b)*|u| and the weights are exact signs
        (num) / 1-over-u capped at 240, TRN2's e4m3 max (den).
  * HOST-side inverse-optimal quantization: the int8 code q is chosen
    so the DEVICE-decoded value is log-nearest to exp(b), merging the
    int8 and mantissa quantizations into one error.
  * PE: bf16 groups cost 512 cols (213 ns); fp8 groups go in PAIRS via
    DoubleRow matmuls (2 j's per partition-cycle). One PSUM [2, 512]:
        row 0 = den = sum exp(b), row 1 = num = sum exp(b)*u.
  * DMA: 9 large triggers on the sync HWDGE queue (trigger issue is
    ~0.6 us each on the queue engine); each trigger carries 2-3 exp
    SEGMENTS of different modes. a/p segment matmuls are DEFERRED 2/3
    triggers so the slow exp engines' latency hides behind the stream.
  * PE warm-up dummies burn the idle pre-stream window (DVFS ramp).

Host: s = num/den, global squash (O(4096) scalar work).
"""

import os
from contextlib import ExitStack

import numpy as np

J = 8192
CAPS = 4096
N_CORES = 8
R = CAPS // N_CORES              # 512 rows (capsules) per core
JG = J // 128                    # 64 j-groups of 128

# Two-level schedule: "|"-separated DMA triggers, each a comma list of
# exp segments "<mode><groups>".
#   a = ACT true exp (bf16)        s = DVE Schraudolph bf16
#   f = DVE Schraudolph e4m3       p = GpSimd Schraudolph e4m3
_SCHED = os.environ.get(
    "KERNEL_SCHED",
    "a2,p2,f4|a2,p2,f4|a3,p2,f4|a3,p2,f4|a3,p2,s4|a3,p2,s4|a2,f4|a2,s4")
SCHED = [[(t[0], int(t[1:])) for t in trig.split(",")]
         for trig in _SCHED.split("|")]
SEGS = [seg for trig in SCHED for seg in trig]
NA = sum(g for m, g in SEGS if m == "a")
NS = sum(g for m, g in SEGS if m == "s")
NF = sum(g for m, g in SEGS if m in "fp")
DEFER = {"a": 2, "p": 3, "s": 0, "f": 0}

S8 = float(os.environ.get("KERNEL_S8", str(5.45 / 127)))
K1_16 = 128.0 / 0.6931471805599453     # 2^7 / ln2  (bf16 bits per unit b)
C16 = 7.0
K2_16 = 127.0 * 128.0 - C16
K1_8 = 8.0 / 0.6931471805599453        # 2^3 / ln2  (e4m3 bits per unit b)
C8 = 0.438
K2_8 = 7.0 * 8.0 - C8
QF_MIN = -112                           # smallest f-tier code: bits(q) >= 0

_CACHED = {}


def _check_cfg():
    assert NA + NS + NF == JG
    assert all(g % 2 == 0 for m, g in SEGS if m in "fp")
    assert NF % 2 == 0


def _seg_meta():
    """Per segment: (trig_idx, mode, groups, col_off, c16_base, p8_base)."""
    meta, off, c16, p8 = [], 0, 0, 0
    for t, trig in enumerate(SCHED):
        for m, g in trig:
            meta.append((t, m, g, off, c16, p8))
            off += g * R
            if m in "as":
                c16 += g
            else:
                p8 += g // 2
    assert off == JG * R and c16 == NA + NS and p8 == NF // 2
    return meta


def _emission_order(meta):
    """Segment MM emission order: s/f inline, a/p deferred by trigger."""
    order, pend = [], []
    n_trig = len(SCHED)
    for t in range(n_trig):
        for i, (ti, m, g, off, c16b, p8b) in enumerate(meta):
            if ti == t:
                if m in "sf":
                    order.append(i)
                else:
                    pend.append(i)
        for i in list(pend):
            ti, m = meta[i][0], meta[i][1]
            if ti <= t - DEFER[m]:
                order.append(i)
                pend.remove(i)
    order.extend(pend)
    assert sorted(order) == list(range(len(meta)))
    return order


def _build_bass():
    import concourse.bass as bass
    import concourse.tile as tile
    from concourse import bacc, mybir

    _check_cfg()
    f32 = mybir.dt.float32
    bf16 = mybir.dt.bfloat16
    i16 = mybir.dt.int16
    i8 = mybir.dt.int8
    f8 = mybir.dt.float8e4

    nc = bacc.Bacc("TRN2", target_bir_lowering=False, debug=False,
                   num_devices=N_CORES)

    bt8_ap = nc.dram_tensor("bt8", [128, JG * R], i8,
                            kind="ExternalInput").ap()
    w16_ap = nc.dram_tensor("w16", [128, 2 * (NA + NS)], bf16,
                            kind="ExternalInput").ap()
    # DoubleRow ldweights needs the pair-dim stride %16B == 0: pad
    # each weight column to 16 bytes.
    w8_ap = nc.dram_tensor("w8", [128, NF // 2, 2, 16], f8,
                           kind="ExternalInput").ap()
    out_ap = nc.dram_tensor("nd_out", [2, R], f32,
                            kind="ExternalOutput").ap()

    meta = _seg_meta()
    order = _emission_order(meta)
    total_mm = (NA + NS) + NF // 2

    with tile.TileContext(nc) as tc, ExitStack() as ctx:
        # every pool holds ALL its tiles at once (total SBUF ~90 KiB)
        # so buffer rotation never gates the DMA stream or an exp engine
        n_trig = len(SCHED)
        n16 = sum(1 for m, g in SEGS if m in "as")
        n8 = sum(1 for m, g in SEGS if m in "fp")
        bpool = ctx.enter_context(tc.tile_pool(name="bl", bufs=n_trig))
        e16p = ctx.enter_context(tc.tile_pool(name="e16", bufs=n16))
        e8p = ctx.enter_context(tc.tile_pool(name="e8", bufs=n8))
        wpool = ctx.enter_context(tc.tile_pool(name="w", bufs=1))
        opool = ctx.enter_context(tc.tile_pool(name="o", bufs=1))
        psum = ctx.enter_context(
            tc.tile_pool(name="ps", bufs=1, space=bass.MemorySpace.PSUM))

        # w on the vector HWDGE queue (DVE idle early): keeps ACT's
        # queue clear so its table load finishes before the first a-data
        # lands.
        w16_sb = wpool.tile([128, 2 * (NA + NS)], bf16)
        nc.scalar.dma_start(w16_sb[:], w16_ap[:, :])
        w8_sb = wpool.tile([128, NF // 2, 2, 16], f8)
        nc.scalar.dma_start(w8_sb[:], w8_ap[:, :, :, :])

        # PE DVFS warm-up in the idle pre-stream window.
        wu = int(os.environ.get("KERNEL_WARMUP_MM", "16"))
        d_ps = dummy = None
        if wu:
            dpool = ctx.enter_context(tc.tile_pool(name="dmy", bufs=1))
            dps = ctx.enter_context(
                tc.tile_pool(name="dps", bufs=1,
                             space=bass.MemorySpace.PSUM))
            dummy = dpool.tile([128, 256], bf16)
            nc.vector.memset(dummy[:], 0.0)
            d_ps = dps.tile([2, 256], f32)
            for _ in range(wu):
                nc.tensor.matmul(d_ps[:, :], dummy[:, 0:2], dummy[:, :],
                                 start=True, stop=True)

        nd_ps = psum.tile([2, R], f32)

        # One DMA trigger per trigger-group; exp per segment.
        e_tiles = {}
        seg_i = 0
        for t, trig in enumerate(SCHED):
            tg = sum(g for _, g in trig)
            t_off = meta[seg_i][3]
            bt = bpool.tile([128, tg * R], i8, tag="bl")
            nc.sync.dma_start(bt[:], bt8_ap[:, t_off:t_off + tg * R])
            b_off = 0
            for m, g in trig:
                if m in "fp":
                    et = e8p.tile([128, g, R], f8, tag="e8")
                else:
                    et = e16p.tile([128, g, R], bf16, tag="e16")
                e_tiles[seg_i] = et
                # small-first halves so the in-order PE starts sooner
                if m in "sf" and g >= 4:
                    halves = [(0, 2), (2, g)]
                else:
                    halves = [(0, g)]
                for lo, hi in halves:
                    e2 = et[:, lo:hi, :].opt()
                    b2 = bt[:, (b_off + lo) * R:(b_off + hi) * R]
                    if m == "a":
                        nc.scalar.activation(
                            e2, b2, mybir.ActivationFunctionType.Exp,
                            scale=S8)
                    elif m == "s":
                        nc.vector.tensor_scalar(
                            out=e2.bitcast(i16), in0=b2,
                            scalar1=S8 * K1_16, scalar2=K2_16,
                            op0=mybir.AluOpType.mult,
                            op1=mybir.AluOpType.add)
                    elif m == "f":
                        nc.vector.tensor_scalar(
                            out=e2.bitcast(i8), in0=b2,
                            scalar1=S8 * K1_8, scalar2=K2_8,
                            op0=mybir.AluOpType.mult,
                            op1=mybir.AluOpType.add)
                    else:
                        nc.gpsimd.tensor_scalar(
                            out=e2.bitcast(i8), in0=b2,
                            scalar1=S8 * K1_8, scalar2=K2_8,
                            op0=mybir.AluOpType.mult,
                            op1=mybir.AluOpType.add)
                b_off += g
                seg_i += 1

        # Matmuls in deferred emission order, one PSUM accumulation
        # group: psum[0,:] += 1-col (den), psum[1,:] += u-col (num).
        mm_idx = 0
        for si in order:
            ti, m, g, off, c16b, p8b = meta[si]
            et = e_tiles[si]
            if m in "as":
                for i in range(g):
                    c = c16b + i
                    nc.tensor.matmul(
                        nd_ps[:, :], w16_sb[:, 2 * c:2 * c + 2],
                        et[:, i, :],
                        start=(mm_idx == 0), stop=(mm_idx == total_mm - 1))
                    mm_idx += 1
            else:
                for i in range(g // 2):
                    p = p8b + i
                    nc.tensor.matmul(
                        nd_ps[:, :], w8_sb[:, p, :, 0:2],
                        et[:, 2 * i:2 * i + 2, :],
                        perf_mode=mybir.MatmulPerfMode.DoubleRow,
                        start=(mm_idx == 0), stop=(mm_idx == total_mm - 1))
                    mm_idx += 1
            if dummy is not None and mm_idx <= 30:
                nc.tensor.matmul(d_ps[:, :], dummy[:, 0:2],
                                 dummy[:, :], start=True, stop=True)
        assert mm_idx == total_mm

        # DMA cannot read PSUM; bounce through SBUF on ACT (idle at the
        # tail; DVE is the loaded engine), output on the sync queue.
        nd_sb = opool.tile([2, R], f32)
        nc.scalar.activation(nd_sb[:], nd_ps[:],
                             mybir.ActivationFunctionType.Identity,
                             scale=1.0)
        nc.sync.dma_start(out_ap[:, :], nd_sb[:])

    nc.compile()
    return nc


def _get_nc():
    if "nc" not in _CACHED:
        _CACHED["nc"] = _build_bass()
    return _CACHED["nc"]


def _img(x):
    """[R, n*128] slot-major -> [128, n*R] partition-major image."""
    r, w = x.shape
    n = w // 128
    return np.ascontiguousarray(
        x.T.reshape(n, 128, r).transpose(1, 0, 2).reshape(128, n * r))


def _grids():
    """Device-decoded value grids for inverse-optimal quantization."""
    import ml_dtypes
    bf16 = ml_dtypes.bfloat16
    e4m3 = ml_dtypes.float8_e4m3fn
    q = np.arange(-127, 128, dtype=np.float32)
    bits16 = np.rint(q * np.float32(S8 * K1_16)
                     + np.float32(K2_16)).astype(np.int16)
    v16 = bits16.view(bf16).astype(np.float64)
    qf = np.arange(QF_MIN, 128, dtype=np.float32)
    bits8 = np.rint(qf * np.float32(S8 * K1_8)
                    + np.float32(K2_8)).astype(np.int8)
    assert bits8.min() >= 0
    v8 = bits8.view(e4m3).astype(np.float64)
    return v16, v8


def _inv_quant(x, v, q0):
    """Per-element code q minimizing log-distance of decode v[q-q0] to
    exp(x). v must be non-decreasing."""
    lv = np.log(np.maximum(v, 1e-300)).astype(np.float32)
    mid = (lv[1:] + lv[:-1]) * np.float32(0.5)
    idx = np.searchsorted(mid, x.astype(np.float32))
    return (idx + q0).astype(np.int8)


def kernel(u_hat: np.ndarray, b: np.ndarray) -> np.ndarray:
    import ml_dtypes
    from concourse import bass_utils

    bf16 = ml_dtypes.bfloat16
    e4m3 = ml_dtypes.float8_e4m3fn
    assert u_hat.shape == (J,) and b.shape == (CAPS, J)
    nc = _get_nc()

    order_u = np.argsort(np.abs(u_hat), kind="stable")
    pool_f = list(order_u[:NF * 128])          # f and p segments
    pool_s = list(order_u[NF * 128:(NF + NS) * 128])
    pool_a = list(order_u[(NF + NS) * 128:])
    pools = {"a": pool_a, "s": pool_s, "f": pool_f, "p": pool_f}

    # slot order = image order = segment-major per SCHED
    jslot = np.empty(J, np.int64)
    pos = 0
    for m, g in SEGS:
        n = g * 128
        jslot[pos:pos + n] = pools[m][:n]
        del pools[m][:n]
        pos += n
    assert pos == J and not pool_f and not pool_s and not pool_a

    v16, v8 = _grids()
    q_all = np.empty((CAPS, J), np.int8)
    w16 = np.empty((128, 2 * (NA + NS)), dtype=bf16)
    w8 = np.zeros((128, NF // 2, 2, 16), dtype=e4m3)
    pos = 0
    c16 = p8 = 0
    for m, g in SEGS:
        n = g * 128
        js = jslot[pos:pos + n]
        cols = slice(pos, pos + n)
        if m == "a":
            q_all[:, cols] = np.clip(
                np.rint(b[:, js] / S8), -127, 127).astype(np.int8)
        elif m == "s":
            q_all[:, cols] = _inv_quant(b[:, js], v16, -127)
        else:
            au = np.abs(u_hat[js]).astype(np.float64)
            q_all[:, cols] = _inv_quant(
                b[:, js] + np.log(np.maximum(au, 1e-300))[None, :],
                v8, QF_MIN)
        if m in "as":
            uu = u_hat[js].astype(bf16).reshape(g, 128)
            for i in range(g):
                w16[:, 2 * (c16 + i)] = 1.0
                w16[:, 2 * (c16 + i) + 1] = uu[i]
            c16 += g
        else:
            au = np.abs(u_hat[js]).reshape(g, 128)
            sg = np.where(u_hat[js] >= 0, 1.0, -1.0).reshape(g, 128)
            wd = np.minimum(1.0 / np.maximum(au, 1e-30), 240.0)
            for i in range(g // 2):
                w8[:, p8 + i, 0, 0] = wd[2 * i].astype(e4m3)
                w8[:, p8 + i, 0, 1] = sg[2 * i].astype(e4m3)
                w8[:, p8 + i, 1, 0] = wd[2 * i + 1].astype(e4m3)
                w8[:, p8 + i, 1, 1] = sg[2 * i + 1].astype(e4m3)
            p8 += g // 2
        pos += n

    in_maps = []
    for i in range(N_CORES):
        rows = slice(i * R, (i + 1) * R)
        in_maps.append({"bt8": _img(q_all[rows]), "w16": w16, "w8": w8})

    res = bass_utils.run_bass_kernel_spmd(
        nc, in_maps, core_ids=list(range(N_CORES)),
        trace=bool(int(os.environ.get("KERNEL_TRACE", "0"))),
    )
    _CACHED["last_results"] = res

    nd = np.stack([r["nd_out"] for r in res.results]).astype(np.float64)
    den = nd[:, 0, :].reshape(-1)
    num = nd[:, 1, :].reshape(-1)
    s = num / den

    s_mag_sq = np.sum(s * s)
    s_mag = np.sqrt(s_mag_sq)
    v = s_mag_sq * s / ((1.0 + s_mag_sq) * s_mag)
    return v.astype(np.float32)
